# revision 1
# baseline (speedup 1.0000x reference)
"""DLRM pairwise-interaction layer on 8 Trainium2 NeuronCores.

Computes, for each batch row b, the strict upper triangle of the Gram matrix
G_b = E_b @ E_b.T where E_b is (27 features, 128 dims), i.e. the reference

    interactions = einsum("bfd,bgd->bfg", E, E);  out = interactions[:, triu_i, triu_j]

Strategy (pure batch data-parallel, 2048 rows/core), bf16 end-to-end on HBM:
  * Host pads features 27 -> 32 (zero cols), casts to bf16, and transposes to
    (128, rows*32) so the contraction dim D=128 lies on SBUF partitions and
    each batch row occupies 32 contiguous columns.
  * 4 batch rows per matmul group: stationary == moving == the group's 128
    contiguous bf16 columns.  One matmul (K=128, M=N=128) computes all 16
    row-pair blocks; the four wanted diagonal 27x27 blocks land at PSUM
    (32q..32q+26, 32q..32q+27) -- engine PSUM access must start at a
    32-aligned partition, which this layout satisfies.  NumWeights==128 +
    bf16 triggers the compiler's fast-weight-load; four matmuls exactly fill
    one 2 KiB PSUM bank.
  * PSUM is managed as two 4-bank tiles (16 matmuls each).  Per tile, 4
    strip-copies (FD=448) extract + cast the diagonal blocks to a compact
    bf16 SBUF tile; tiles alternate VectorE/ScalarE so the engines never
    touch the same PSUM banks concurrently.
  * Input DMAs are the only DMAs on the Sync queue (HWDGE is FIFO per
    issuing engine -- mixing output DMAs in would stall later input DMAs
    behind the output DMAs' copy-completion waits).  Output DMA is one
    contiguous full-128-partition transfer per chunk issued from ScalarE's
    HWDGE ring; the host drops the 5 pad partitions per strip and does the
    cheap (B, 27, 28) -> (B, 351) triangle gather in fp32.
"""

import numpy as np

B = 16384
F = 27
FP = 32                     # padded feature count (32-aligned PSUM strips)
FO = 28                     # output block width (27 real + 1 pad col)
D = 128
NCORES = 8
BLOC = B // NCORES          # 2048 batch rows per core
BCHUNK = 256                # batch rows per pipeline chunk
NCHUNK = BLOC // BCHUNK     # 8
NPASS = 4                   # psum passes per chunk (16 groups = 64 rows each)
CHUNK_COLS = BCHUNK * FP    # 8192 bf16 per partition per chunk
PAD_COLS = 128              # stationary tail for the last group
ET_COLS = BLOC * FP + PAD_COLS  # 65664

_TRIU_I, _TRIU_J = np.triu_indices(F, k=1)

_compiled = None


def _build():
    import concourse.bacc as bacc
    import concourse.mybir as mybir
    from concourse.tile import TileContext

    f32 = mybir.dt.float32
    bf16 = mybir.dt.bfloat16
    nc = bacc.Bacc(None, target_bir_lowering=False)

    et = nc.dram_tensor("et", [D, ET_COLS], bf16, kind="ExternalInput")
    y = nc.dram_tensor("y", [NCHUNK, D, NPASS, 4, 4, FO], bf16,
                       kind="ExternalOutput")

    with TileContext(nc) as tc:
        with (
            tc.tile_pool(name="inp", bufs=4) as inp,
            tc.tile_pool(name="outp", bufs=3) as outp,
            tc.tile_pool(name="psum", bufs=2, space="PSUM") as psum,
        ):
            for c in range(NCHUNK):
                in_t = inp.tile([D, CHUNK_COLS + PAD_COLS], bf16)
                nc.sync.dma_start(
                    in_t[:, :],
                    et[:, c * CHUNK_COLS:c * CHUNK_COLS + CHUNK_COLS + PAD_COLS],
                )
                out_t = outp.tile([D, NPASS, 4, 4, FO], bf16)
                for hh in range(NPASS):
                    ps = psum.tile([D, 4, 4, 128], f32)  # 4 banks
                    for bk in range(4):
                        for s in range(4):
                            g = 16 * hh + 4 * bk + s
                            ein = in_t[:, 128 * g:128 * g + 128]
                            nc.tensor.matmul(ps[:, bk, s, :], ein, ein,
                                             start=True, stop=True)
                    for q in range(4):
                        src = ps[32 * q:32 * q + F, :, :, 32 * q:32 * q + FO]
                        dst = out_t[32 * q:32 * q + F, hh, :, :, :]
                        if hh % 2 == 0:
                            nc.vector.tensor_copy(dst, src)
                        else:
                            nc.scalar.copy(dst, src)
                nc.scalar.dma_start(y[c], out_t[:, :, :, :, :])

    nc.compile()
    return nc


def _get_compiled():
    global _compiled
    if _compiled is None:
        _compiled = _build()
    return _compiled


def _prep_inputs(embeddings: np.ndarray):
    """Full (B, 27, 128) fp32 -> per-core padded bf16 (128, ET_COLS)."""
    import ml_dtypes

    bf16 = ml_dtypes.bfloat16
    e = np.asarray(embeddings, dtype=np.float32)
    # (D, B, F) bf16
    eT = np.ascontiguousarray(e.transpose(2, 0, 1)).astype(bf16)
    padded = np.zeros((D, B, FP), dtype=bf16)
    padded[:, :, :F] = eT
    in_maps = []
    for c in range(NCORES):
        etc = np.zeros((D, ET_COLS), dtype=bf16)
        etc[:, :BLOC * FP] = padded[:, c * BLOC:(c + 1) * BLOC, :].reshape(
            D, BLOC * FP
        )
        in_maps.append({"et": etc})
    return in_maps


def _decode_core(yv: np.ndarray) -> np.ndarray:
    """(NCHUNK, 128, NPASS, 4, 4, FO) bf16 -> (BLOC, 351) fp32."""
    g = np.asarray(yv).astype(np.float32)
    g = g.reshape(NCHUNK, 4, 32, NPASS, 4, 4, FO)[:, :, :F]
    # row = c*256 + 64*hh + 16*bk + 4*s + q ; partition 32q+i holds (i, j)
    g = g.transpose(0, 3, 4, 5, 1, 2, 6).reshape(BLOC, F, FO)
    return g[:, _TRIU_I, _TRIU_J]


def kernel(embeddings: np.ndarray) -> np.ndarray:
    from concourse.bass_utils import run_bass_kernel_spmd

    nc = _get_compiled()
    in_maps = _prep_inputs(embeddings)
    res = run_bass_kernel_spmd(nc, in_maps, core_ids=list(range(NCORES)))

    out = np.empty((B, len(_TRIU_I)), dtype=np.float32)
    for c in range(NCORES):
        out[c * BLOC:(c + 1) * BLOC] = _decode_core(res.results[c]["y"])
    return out

